# revision 1
# baseline (speedup 1.0000x reference)
"""Raw-Bacc (no TileContext) CenterLoss kernel.

Per core (128 batch rows):
  SP queue:  DMA labels [128,1] int32 -> SBUF
  ACT queue: DMA x [128,512] -> SBUF
  Pool:      indirect DMA gathers centers[labels] rows into SBUF
  DVE:       diff = x - c, square, row-reduce, clip to [1e-12, 1e12]
  PE:        ones-matmul reduces the 128 partition values to one scalar
  DVE:       PSUM -> SBUF copy;  SP: DMA scalar out
Host sums the 8 per-core partials (the all-reduce) and divides by B.
"""

import numpy as np

_BATCH = 1024
_FEAT = 512
_NCLASSES = 10000
_NCORES = 8
_ROWS = _BATCH // _NCORES  # 128
_P = 128

_state = {}


def _build_nc_raw(decoy=False):
    import concourse.bass as bass
    import concourse.mybir as mybir
    from concourse import bacc

    f32 = mybir.dt.float32
    i32 = mybir.dt.int32
    nc = bacc.Bacc("TRN2", target_bir_lowering=False, debug=False)
    x_d = nc.dram_tensor("x", [_ROWS, _FEAT], f32, kind="ExternalInput").ap()
    labels_d = nc.dram_tensor("labels", [_ROWS, 1], i32, kind="ExternalInput").ap()
    centers_d = nc.dram_tensor(
        "centers", [_NCLASSES, _FEAT], f32, kind="ExternalInput"
    ).ap()
    out_d = nc.dram_tensor("out", [1, 1], f32, kind="ExternalOutput").ap()

    from contextlib import ExitStack

    with ExitStack() as _es:
        labels_t = _es.enter_context(nc.sbuf_tensor("labels_t", [_ROWS, 1], i32))
        decoy_t = _es.enter_context(nc.sbuf_tensor("decoy_t", [1, 4], f32))
        x_t = _es.enter_context(nc.sbuf_tensor("x_t", [_P, _FEAT], f32))
        c_t = _es.enter_context(nc.sbuf_tensor("c_t", [_P, _FEAT], f32))
        diff_t = _es.enter_context(nc.sbuf_tensor("diff_t", [_P, _FEAT], f32))
        sq_t = _es.enter_context(nc.sbuf_tensor("sq_t", [_P, _FEAT], f32))
        xm2_t = _es.enter_context(nc.sbuf_tensor("xm2_t", [_P, _FEAT], f32))
        u_t = _es.enter_context(nc.sbuf_tensor("u_t", [_P, _FEAT], f32))
        xx_t = _es.enter_context(nc.sbuf_tensor("xx_t", [_P, 1], f32))
        s1_t = _es.enter_context(nc.sbuf_tensor("s1_t", [_P, 1], f32))
        d_t = _es.enter_context(nc.sbuf_tensor("d_t", [_P, 1], f32))
        dc_t = _es.enter_context(nc.sbuf_tensor("dc_t", [_P, 1], f32))
        ones_t = _es.enter_context(nc.sbuf_tensor("ones_t", [_P, 1], f32))
        res_t = _es.enter_context(nc.sbuf_tensor("res_t", [1, 1], f32))
        acc_t = _es.enter_context(nc.psum_tensor("acc_t", [1, 1], f32))
        lab_sem = _es.enter_context(nc.semaphore("lab_sem"))
        decoy_sem = _es.enter_context(nc.semaphore("decoy_sem"))
        x_sem = _es.enter_context(nc.semaphore("x_sem"))
        c_sem = _es.enter_context(nc.semaphore("c_sem"))
        dve_sem = _es.enter_context(nc.semaphore("dve_sem"))
        m_sem = _es.enter_context(nc.semaphore("m_sem"))
        o_sem = _es.enter_context(nc.semaphore("o_sem"))
        # labels on the SP HWDGE queue so the gather can start ASAP
        nc.sync.dma_start(labels_t.ap(), labels_d).then_inc(lab_sem, 16)
        # x on the ACT HWDGE queue, overlaps the gather
        nc.scalar.dma_start(x_t.ap(), x_d).then_inc(x_sem, 16)
        nc.vector.memset(ones_t.ap(), 1.0)

        if decoy:
            # tiny SWDGE DMA warms the Pool dynamic-DMA path so the real
            # gather's ucode drain is cheap
            nc.gpsimd.dma_start(decoy_t.ap(), centers_d[0:1, 0:4]).then_inc(
                decoy_sem, 16
            )

        nc.gpsimd.wait_ge(lab_sem, 16)
        nc.gpsimd.indirect_dma_start(
            out=c_t.ap(),
            out_offset=None,
            in_=centers_d,
            in_offset=bass.IndirectOffsetOnAxis(ap=labels_t.ap()[:, :1], axis=0),
        ).then_inc(c_sem, 16)
        if decoy:
            nc.gpsimd.wait_ge(decoy_sem, 16)

        # hidden under the gather: xx = rowsum(x*x), xm2 = -2x
        nc.vector.wait_ge(x_sem, 16)
        nc.vector.scalar_tensor_tensor(
            out=sq_t.ap(), in0=x_t.ap(), scalar=1.0, in1=x_t.ap(),
            op0=mybir.AluOpType.mult, op1=mybir.AluOpType.mult,
            accum_out=xx_t.ap(),
        ).then_inc(dve_sem, 1)
        nc.vector.tensor_scalar_mul(xm2_t.ap(), x_t.ap(), -2.0).then_inc(dve_sem, 1)
        # post-gather: d = rowsum(c*(c-2x)) + xx  (= rowsum((c-x)^2))
        nc.vector.wait_ge(c_sem, 16)
        nc.vector.wait_ge(dve_sem, 2)
        nc.vector.tensor_tensor(
            out=u_t.ap(), in0=c_t.ap(), in1=xm2_t.ap(), op=mybir.AluOpType.add
        ).then_inc(dve_sem, 1)
        nc.vector.wait_ge(dve_sem, 3)
        nc.vector.scalar_tensor_tensor(
            out=diff_t.ap(), in0=u_t.ap(), scalar=0.0, in1=c_t.ap(),
            op0=mybir.AluOpType.add, op1=mybir.AluOpType.mult,
            accum_out=s1_t.ap(),
        ).then_inc(dve_sem, 1)
        nc.vector.wait_ge(dve_sem, 4)
        nc.vector.tensor_tensor(
            out=d_t.ap(), in0=s1_t.ap(), in1=xx_t.ap(), op=mybir.AluOpType.add
        ).then_inc(dve_sem, 1)
        nc.vector.wait_ge(dve_sem, 5)
        nc.vector.tensor_scalar(
            out=dc_t.ap(),
            in0=d_t.ap(),
            scalar1=1e-12,
            scalar2=1e12,
            op0=mybir.AluOpType.max,
            op1=mybir.AluOpType.min,
        ).then_inc(dve_sem, 1)

        nc.tensor.wait_ge(dve_sem, 6)
        nc.tensor.matmul(
            acc_t.ap(), lhsT=dc_t.ap(), rhs=ones_t.ap(), start=True, stop=True
        ).then_inc(m_sem, 1)

        nc.vector.wait_ge(m_sem, 1)
        nc.vector.tensor_copy(out=res_t.ap(), in_=acc_t.ap()).then_inc(dve_sem, 1)

        nc.sync.wait_ge(dve_sem, 7)
        nc.sync.dma_start(out_d, res_t.ap()).then_inc(o_sem, 16)

    nc.compile()
    return nc


def _run(x, labels, centers, trace=False, decoy=False):
    from concourse.bass_utils import run_bass_kernel_spmd

    key = ("nc", decoy)
    if key not in _state:
        _state[key] = _build_nc_raw(decoy=decoy)
    nc = _state[key]

    x = np.ascontiguousarray(np.asarray(x, dtype=np.float32)).reshape(
        _NCORES, _ROWS, _FEAT
    )
    lab = (
        np.ascontiguousarray(np.asarray(labels))
        .astype(np.int32)
        .reshape(_NCORES, _ROWS, 1)
    )
    cen = np.ascontiguousarray(np.asarray(centers, dtype=np.float32))
    in_maps = [{"x": x[i], "labels": lab[i], "centers": cen} for i in range(_NCORES)]
    res = run_bass_kernel_spmd(nc, in_maps, core_ids=list(range(_NCORES)), trace=trace)
    total = 0.0
    for r in res.results:
        total += float(r["out"][0, 0])
    loss = total / _BATCH + (_NCLASSES - 1) * 1e-12
    return np.float32(loss), res


def kernel(x, labels, centers):
    loss, _ = _run(x, labels, centers, trace=False, decoy=True)
    return loss



# revision 11
# speedup vs baseline: 1.0256x; 1.0256x over previous
"""Raw-Bacc CenterLoss kernel, v2.

The masked distmat sum reduces to: loss = mean_b ||x_b - c_{label_b}||^2
(clip only affects the 9999 zero entries per row -> host-side constant).

Per core (128 batch rows), the device computes two [128,1] partials:
  col0: s1_p = sum_f x[p,f]^2 - 2*sum_f x[p,f]*c[p,f]   (DVE, 2 ttr passes)
  col1: s2_p = sum_f c[p,f]^2                           (ACT square+accum)
where c = centers[labels] via SWDGE indirect gather.

Timeline per core:
  SP (pre-barrier): labels [128,1] i32 DMA -> SBUF     (hoisted before the
      all-engine barrier so its ~2us completion latency overlaps startup)
  ACT: x [128,512] DMA; dummy activation to force the Square table load
      off the critical path
  DVE (hidden under gather): xx = rowsum(x*x)
  Pool: indirect gather centers[labels] -> c
  DVE: ttr  s1 = xx + rowsum((c*x)*-2)   ||  ACT: s2 = rowsum(square(c))
  SP: DMA [128,2] partials out
Host: clip per-row dist, sum 1024 partials, /B, + clip compensation.
"""

import os

import numpy as np

_BATCH = 1024
_FEAT = 512
_NCLASSES = 10000
_NCORES = 8
_ROWS = _BATCH // _NCORES  # 128
_P = 128

_state = {}

# knobs (A/B testable via env; defaults are the shipping config)
_PREBARRIER = os.environ.get("K_PREBARRIER", "1") == "1"
_ACT_WARMUP = os.environ.get("K_ACT_WARMUP", "1") == "1"
_USE_ACT = os.environ.get("K_USE_ACT", "1") == "1"


def _build_nc_raw():
    import concourse.bass as bass
    import concourse.mybir as mybir
    from concourse import bacc

    f32 = mybir.dt.float32
    i32 = mybir.dt.int32
    Alu = mybir.AluOpType
    Act = mybir.ActivationFunctionType

    nc = bacc.Bacc("TRN2", target_bir_lowering=False, debug=False)
    x_d = nc.dram_tensor("x", [_ROWS, _FEAT], f32, kind="ExternalInput").ap()
    labels_d = nc.dram_tensor("labels", [_ROWS, 1], i32, kind="ExternalInput").ap()
    centers_d = nc.dram_tensor(
        "centers", [_NCLASSES, _FEAT], f32, kind="ExternalInput"
    ).ap()
    out_d = nc.dram_tensor("out", [_P, 2], f32, kind="ExternalOutput").ap()

    from contextlib import ExitStack

    with ExitStack() as _es:
        ec = _es.enter_context
        labels_t = ec(nc.sbuf_tensor("labels_t", [_ROWS, 1], i32))
        x_t = ec(nc.sbuf_tensor("x_t", [_P, _FEAT], f32))
        c_t = ec(nc.sbuf_tensor("c_t", [_P, _FEAT], f32))
        junk_dve = ec(nc.sbuf_tensor("junk_dve", [_P, _FEAT], f32))
        junk_dve2 = ec(nc.sbuf_tensor("junk_dve2", [_P, _FEAT], f32))
        junk_act = ec(nc.sbuf_tensor("junk_act", [_P, _FEAT], f32))
        warm_t = ec(nc.sbuf_tensor("warm_t", [_P, 1], f32))
        xx_t = ec(nc.sbuf_tensor("xx_t", [_P, 1], f32))
        sxc_t = ec(nc.sbuf_tensor("sxc_t", [_P, 1], f32))
        part_t = ec(nc.sbuf_tensor("part_t", [_P, 2], f32))
        lab_sem = ec(nc.semaphore("lab_sem"))
        x_sem = ec(nc.semaphore("x_sem"))
        c_sem = ec(nc.semaphore("c_sem"))
        dve_sem = ec(nc.semaphore("dve_sem"))
        xx_sem = ec(nc.semaphore("xx_sem"))
        act_sem = ec(nc.semaphore("act_sem"))
        o_sem = ec(nc.semaphore("o_sem"))

        # labels on the SP HWDGE queue; hoisted pre-barrier below
        lab_dma = nc.sync.dma_start(labels_t.ap(), labels_d)
        lab_dma.then_inc(lab_sem, 16)
        # x on the ACT HWDGE queue
        nc.scalar.dma_start(x_t.ap(), x_d).then_inc(x_sem, 16)
        if _USE_ACT and _ACT_WARMUP:
            # tiny activation with no data deps: forces the Square table
            # load (~1.3us) to happen during the gather window. Reads the
            # framework's const-zero AP (initialized in the preamble).
            const0 = nc.const_aps.aps[(f32, 0.0)]
            nc.scalar.activation(out=warm_t.ap(), in_=const0, func=Act.Square)

        # gather c = centers[labels]
        nc.gpsimd.wait_ge(lab_sem, 16)
        nc.gpsimd.indirect_dma_start(
            out=c_t.ap(),
            out_offset=None,
            in_=centers_d,
            in_offset=bass.IndirectOffsetOnAxis(ap=labels_t.ap()[:, :1], axis=0),
        ).then_inc(c_sem, 16)

        # hidden under the gather: xx = rowsum(x*x)
        nc.vector.wait_ge(x_sem, 16)
        nc.vector.scalar_tensor_tensor(
            out=junk_dve.ap(),
            in0=x_t.ap(),
            scalar=1.0,
            in1=x_t.ap(),
            op0=Alu.mult,
            op1=Alu.mult,
            accum_out=xx_t.ap(),
        ).then_inc(xx_sem, 1)

        # post-gather: sxc = rowsum(-2*c*x), then s1 = sxc + xx  (DVE)
        nc.vector.wait_ge(c_sem, 16)
        nc.vector.scalar_tensor_tensor(
            out=junk_dve2.ap(),
            in0=c_t.ap(),
            scalar=-2.0,
            in1=x_t.ap(),
            op0=Alu.mult,
            op1=Alu.mult,
            accum_out=sxc_t.ap(),
        ).then_inc(dve_sem, 1)
        nc.vector.wait_ge(xx_sem, 1)
        nc.vector.wait_ge(dve_sem, 1)
        nc.vector.tensor_tensor(
            out=part_t.ap()[:, 0:1],
            in0=sxc_t.ap(),
            in1=xx_t.ap(),
            op=Alu.add,
        ).then_inc(dve_sem, 1)

        if _USE_ACT:
            # post-gather: s2 = rowsum(c^2)  (ACT, parallel with DVE)
            nc.scalar.wait_ge(c_sem, 16)
            nc.scalar.activation(
                out=junk_act.ap(),
                in_=c_t.ap(),
                func=Act.Square,
                accum_out=part_t.ap()[:, 1:2],
            ).then_inc(act_sem, 1)
        else:
            nc.vector.wait_ge(xx_sem, 1)
            nc.vector.scalar_tensor_tensor(
                out=junk_dve.ap(),
                in0=c_t.ap(),
                scalar=1.0,
                in1=c_t.ap(),
                op0=Alu.mult,
                op1=Alu.mult,
                accum_out=part_t.ap()[:, 1:2],
            ).then_inc(act_sem, 1)

        nc.sync.wait_ge(dve_sem, 2)
        nc.sync.wait_ge(act_sem, 1)
        nc.sync.dma_start(out_d, part_t.ap()).then_inc(o_sem, 16)

    if _PREBARRIER:
        # hoist the labels DMA ahead of the all-engine start barrier: insert
        # it right after SP's barrier-arrival drain (which has already bumped
        # the barrier sem, so this does not delay other engines) and before
        # SP's barrier release wait.
        entry = nc.main_func.blocks[0]
        insts = entry.instructions
        li = lab_dma.ins
        sp = mybir.EngineType.SP
        sp_drain_idx = None
        for i, ins in enumerate(insts):
            if isinstance(ins, mybir.InstDrain) and ins.engine == sp:
                sp_drain_idx = i
                break
        if sp_drain_idx is not None and li in insts:
            old_idx = insts.index(li)
            if old_idx > sp_drain_idx:
                insts.remove(li)
                insts.insert(sp_drain_idx + 1, li)

    nc.compile()
    return nc


def _get_nc():
    if "nc" not in _state:
        _state["nc"] = _build_nc_raw()
    return _state["nc"]


def _postprocess(partials):
    """partials: list of [128,2] f32 arrays, one per core."""
    total = 0.0
    for p in partials:
        d = p[:, 0].astype(np.float64) + p[:, 1].astype(np.float64)
        d = np.clip(d, 1e-12, 1e12)
        total += float(d.sum())
    loss = total / _BATCH + (_NCLASSES - 1) * 1e-12
    return np.float32(loss)


def _run(x, labels, centers, trace=False):
    from concourse.bass_utils import run_bass_kernel_spmd

    nc = _get_nc()

    x = np.ascontiguousarray(np.asarray(x, dtype=np.float32)).reshape(
        _NCORES, _ROWS, _FEAT
    )
    lab = (
        np.ascontiguousarray(np.asarray(labels))
        .astype(np.int32)
        .reshape(_NCORES, _ROWS, 1)
    )
    cen = np.ascontiguousarray(np.asarray(centers, dtype=np.float32))
    in_maps = [{"x": x[i], "labels": lab[i], "centers": cen} for i in range(_NCORES)]
    res = run_bass_kernel_spmd(nc, in_maps, core_ids=list(range(_NCORES)), trace=trace)
    loss = _postprocess([r["out"] for r in res.results])
    return loss, res


def kernel(x, labels, centers):
    loss, _ = _run(x, labels, centers, trace=False)
    return loss


# revision 13
# speedup vs baseline: 1.0464x; 1.0203x over previous
"""Raw-Bacc CenterLoss kernel, v2.

The masked distmat sum reduces to: loss = mean_b ||x_b - c_{label_b}||^2
(clip only affects the 9999 zero entries per row -> host-side constant).

Per core (128 batch rows), the device computes two [128,1] partials:
  col0: s1_p = sum_f x[p,f]^2 - 2*sum_f x[p,f]*c[p,f]   (DVE, 2 ttr passes)
  col1: s2_p = sum_f c[p,f]^2                           (ACT square+accum)
where c = centers[labels] via SWDGE indirect gather.

Timeline per core:
  SP (pre-barrier): labels [128,1] i32 DMA -> SBUF     (hoisted before the
      all-engine barrier so its ~2us completion latency overlaps startup)
  ACT: x [128,512] DMA; dummy activation to force the Square table load
      off the critical path
  DVE (hidden under gather): xx = rowsum(x*x)
  Pool: indirect gather centers[labels] -> c
  DVE: ttr  s1 = xx + rowsum((c*x)*-2)   ||  ACT: s2 = rowsum(square(c))
  SP: DMA [128,2] partials out
Host: clip per-row dist, sum 1024 partials, /B, + clip compensation.
"""

import os

import numpy as np

_BATCH = 1024
_FEAT = 512
_NCLASSES = 10000
_NCORES = 8
_ROWS = _BATCH // _NCORES  # 128
_P = 128

_state = {}

# knobs (A/B testable via env; defaults are the shipping config)
_PREBARRIER = os.environ.get("K_PREBARRIER", "1") == "1"
_ACT_WARMUP = os.environ.get("K_ACT_WARMUP", "1") == "1"
_USE_ACT = os.environ.get("K_USE_ACT", "1") == "1"


def _build_nc_raw():
    import concourse.bass as bass
    import concourse.mybir as mybir
    from concourse import bacc

    f32 = mybir.dt.float32
    i32 = mybir.dt.int32
    Alu = mybir.AluOpType
    Act = mybir.ActivationFunctionType

    nc = bacc.Bacc("TRN2", target_bir_lowering=False, debug=False)
    x_d = nc.dram_tensor("x", [_ROWS, _FEAT], f32, kind="ExternalInput").ap()
    labels_d = nc.dram_tensor("labels", [_ROWS, 1], i32, kind="ExternalInput").ap()
    centers_d = nc.dram_tensor(
        "centers", [_NCLASSES, _FEAT], f32, kind="ExternalInput"
    ).ap()
    out_d = nc.dram_tensor("out", [_P, 2], f32, kind="ExternalOutput").ap()

    from contextlib import ExitStack

    with ExitStack() as _es:
        ec = _es.enter_context
        labels_t = ec(nc.sbuf_tensor("labels_t", [_ROWS, 1], i32))
        x_t = ec(nc.sbuf_tensor("x_t", [_P, _FEAT], f32))
        c_t = ec(nc.sbuf_tensor("c_t", [_P, _FEAT], f32))
        junk_dve = ec(nc.sbuf_tensor("junk_dve", [_P, _FEAT], f32))
        junk_dve2 = ec(nc.sbuf_tensor("junk_dve2", [_P, _FEAT], f32))
        junk_act = ec(nc.sbuf_tensor("junk_act", [_P, _FEAT], f32))
        warm_t = ec(nc.sbuf_tensor("warm_t", [_P, 1], f32))
        xx_t = ec(nc.sbuf_tensor("xx_t", [_P, 1], f32))
        sxc_t = ec(nc.sbuf_tensor("sxc_t", [_P, 1], f32))
        part_t = ec(nc.sbuf_tensor("part_t", [_P, 2], f32))
        lab_sem = ec(nc.semaphore("lab_sem"))
        x_sem = ec(nc.semaphore("x_sem"))
        c_sem = ec(nc.semaphore("c_sem"))
        dve_sem = ec(nc.semaphore("dve_sem"))
        xx_sem = ec(nc.semaphore("xx_sem"))
        act_sem = ec(nc.semaphore("act_sem"))
        o_sem = ec(nc.semaphore("o_sem"))

        # labels then x, both on the SP HWDGE ring (hoisted pre-barrier
        # below). Per-engine ring FIFO guarantees the labels descriptors
        # complete before x's start, so the tiny labels spray is never
        # delayed by x's bulk traffic.
        lab_dma = nc.sync.dma_start(labels_t.ap(), labels_d)
        lab_dma.then_inc(lab_sem, 16)
        x_dma = nc.sync.dma_start(x_t.ap(), x_d)
        x_dma.then_inc(x_sem, 16)
        if _USE_ACT and _ACT_WARMUP:
            # tiny activation with no data deps: forces the Square table
            # load (~1.3us) to happen during the gather window. Reads the
            # framework's const-zero AP (initialized in the preamble).
            const0 = nc.const_aps.aps[(f32, 0.0)]
            nc.scalar.activation(out=warm_t.ap(), in_=const0, func=Act.Square)

        # gather c = centers[labels]
        nc.gpsimd.wait_ge(lab_sem, 16)
        nc.gpsimd.indirect_dma_start(
            out=c_t.ap(),
            out_offset=None,
            in_=centers_d,
            in_offset=bass.IndirectOffsetOnAxis(ap=labels_t.ap()[:, :1], axis=0),
        ).then_inc(c_sem, 16)

        # hidden under the gather: xx = rowsum(x*x)
        nc.vector.wait_ge(x_sem, 16)
        nc.vector.scalar_tensor_tensor(
            out=junk_dve.ap(),
            in0=x_t.ap(),
            scalar=1.0,
            in1=x_t.ap(),
            op0=Alu.mult,
            op1=Alu.mult,
            accum_out=xx_t.ap(),
        ).then_inc(xx_sem, 1)

        # post-gather: sxc = rowsum(-2*c*x), then s1 = sxc + xx  (DVE)
        nc.vector.wait_ge(c_sem, 16)
        nc.vector.scalar_tensor_tensor(
            out=junk_dve2.ap(),
            in0=c_t.ap(),
            scalar=-2.0,
            in1=x_t.ap(),
            op0=Alu.mult,
            op1=Alu.mult,
            accum_out=sxc_t.ap(),
        ).then_inc(dve_sem, 1)
        nc.vector.wait_ge(xx_sem, 1)
        nc.vector.wait_ge(dve_sem, 1)
        nc.vector.tensor_tensor(
            out=part_t.ap()[:, 0:1],
            in0=sxc_t.ap(),
            in1=xx_t.ap(),
            op=Alu.add,
        ).then_inc(dve_sem, 1)

        if _USE_ACT:
            # post-gather: s2 = rowsum(c^2)  (ACT, parallel with DVE)
            nc.scalar.wait_ge(c_sem, 16)
            nc.scalar.activation(
                out=junk_act.ap(),
                in_=c_t.ap(),
                func=Act.Square,
                accum_out=part_t.ap()[:, 1:2],
            ).then_inc(act_sem, 1)
        else:
            nc.vector.wait_ge(xx_sem, 1)
            nc.vector.scalar_tensor_tensor(
                out=junk_dve.ap(),
                in0=c_t.ap(),
                scalar=1.0,
                in1=c_t.ap(),
                op0=Alu.mult,
                op1=Alu.mult,
                accum_out=part_t.ap()[:, 1:2],
            ).then_inc(act_sem, 1)

        nc.sync.wait_ge(dve_sem, 2)
        nc.sync.wait_ge(act_sem, 1)
        nc.sync.dma_start(out_d, part_t.ap()).then_inc(o_sem, 16)

    if _PREBARRIER:
        # hoist the labels+x DMAs ahead of the all-engine start barrier:
        # insert them right after SP's barrier-arrival drain (which has
        # already bumped the barrier sem, so this does not delay other
        # engines) and before SP's barrier release wait.
        entry = nc.main_func.blocks[0]
        insts = entry.instructions
        sp = mybir.EngineType.SP
        sp_drain_idx = None
        for i, ins in enumerate(insts):
            if isinstance(ins, mybir.InstDrain) and ins.engine == sp:
                sp_drain_idx = i
                break
        if sp_drain_idx is not None:
            for mv in (x_dma.ins, lab_dma.ins):  # reversed: labels ends first
                if mv in insts and insts.index(mv) > sp_drain_idx:
                    insts.remove(mv)
                    insts.insert(sp_drain_idx + 1, mv)

    nc.compile()
    return nc


def _get_nc():
    if "nc" not in _state:
        _state["nc"] = _build_nc_raw()
    return _state["nc"]


def _postprocess(partials):
    """partials: list of [128,2] f32 arrays, one per core."""
    total = 0.0
    for p in partials:
        d = p[:, 0].astype(np.float64) + p[:, 1].astype(np.float64)
        d = np.clip(d, 1e-12, 1e12)
        total += float(d.sum())
    loss = total / _BATCH + (_NCLASSES - 1) * 1e-12
    return np.float32(loss)


def _run(x, labels, centers, trace=False):
    from concourse.bass_utils import run_bass_kernel_spmd

    nc = _get_nc()

    x = np.ascontiguousarray(np.asarray(x, dtype=np.float32)).reshape(
        _NCORES, _ROWS, _FEAT
    )
    lab = (
        np.ascontiguousarray(np.asarray(labels))
        .astype(np.int32)
        .reshape(_NCORES, _ROWS, 1)
    )
    cen = np.ascontiguousarray(np.asarray(centers, dtype=np.float32))
    in_maps = [{"x": x[i], "labels": lab[i], "centers": cen} for i in range(_NCORES)]
    res = run_bass_kernel_spmd(nc, in_maps, core_ids=list(range(_NCORES)), trace=trace)
    loss = _postprocess([r["out"] for r in res.results])
    return loss, res


def kernel(x, labels, centers):
    loss, _ = _run(x, labels, centers, trace=False)
    return loss
